# revision 1
# baseline (speedup 1.0000x reference)
"""AttentionBlock (GroupNorm32 + 1x1conv QKV + MHA + 1x1conv proj + residual)
on 8 Trainium2 NeuronCores, data-parallel over batch (1 batch item / core).

Layouts (per core, batch item b):
  x, h:   [c, n]   c=512 channels as 4 tiles of 128 partitions, n=H*W=1024 free
  q, k:   [d, n]   from  qkv = Wqk @ h        (o-channel on partitions)
  v^T:    [n, d]   from  (Wv h)^T = h^T Wv^T  (so the o-matmul needs no transpose),
                   with a ones-column appended so the o-matmul also emits softmax sums
  S^T:    [m, n] = k^T q per head ([128, 1024] fp32 PSUM, double-buffered)
  P~^T  = exp(scale * S^T)  (no max subtraction: |scale*S| <~ 6 for this problem)
  o''   = [v^T | 1]^T @ P~^T  -> rows 0:64 = unnormalized o, row 64 = softmax sums
  o     = o'' * bcast(1/sums)  (reciprocal_approx_fast + stride-0 DMA broadcast
                                bounced through a DRAM scratch row)
  out   = Wp @ o + I @ x_bf16 + bias'  (residual fused into the proj matmul;
                                        v-bias folded into bias' on the host)

Engine split: PE matmuls; ScalarE = exp (the attention-phase bottleneck, fed
continuously from ~t=12us by interleaving qk/vT/tail work into the per-pair
loops) plus PSUM evacuations in phases where it is idle; VectorE = groupnorm
stats/apply, qk/vT evacuation, reciprocal, o-normalize.
"""

import numpy as np
import ml_dtypes

B, C, HH, WW = 8, 512, 32, 32
N = HH * WW            # 1024
NUM_HEADS = 8
HD = C // NUM_HEADS    # 64
NUM_GROUPS = 32
GS = C // NUM_GROUPS   # 16 channels / group
EPS = 1e-5
SCALE = HD ** -0.5
CT = 4                 # channel tiles of 128
BF16 = ml_dtypes.bfloat16

_CACHE = {}


def _build_nc(dump=False):
    from contextlib import ExitStack

    import concourse.bacc as bacc
    import concourse.mybir as mybir
    import concourse.tile as tile

    f32 = mybir.dt.float32
    f32r = mybir.dt.float32r
    bf16 = mybir.dt.bfloat16
    AF = mybir.ActivationFunctionType
    OP = mybir.AluOpType

    nc = bacc.Bacc("TRN2", target_bir_lowering=False, debug=False)

    # ---- DRAM parameters ----
    x_d = nc.dram_tensor("x", [CT, 128, 2, 512], f32, kind="ExternalInput")
    wqk_d = nc.dram_tensor("wqkT", [CT, 128, 1024], bf16, kind="ExternalInput")
    wv_d = nc.dram_tensor("wvT", [CT, 128, 512], bf16, kind="ExternalInput")
    wp_d = nc.dram_tensor("wpT", [CT, 128, 512], bf16, kind="ExternalInput")
    qkb_d = nc.dram_tensor("qkbias", [128, 8], f32, kind="ExternalInput")
    pb_d = nc.dram_tensor("pbias", [128, 4], f32, kind="ExternalInput")
    gnw_d = nc.dram_tensor("gnw", [128, 4], f32, kind="ExternalInput")
    gnb_d = nc.dram_tensor("gnb", [128, 4], f32, kind="ExternalInput")
    g_d = nc.dram_tensor("gmat", [128, 4, 8], f32, kind="ExternalInput")
    r_d = nc.dram_tensor("rmat", [8, 4, 128], f32, kind="ExternalInput")
    id_d = nc.dram_tensor("ident", [128, 128], bf16, kind="ExternalInput")
    out_d = nc.dram_tensor("out", [CT, 128, 2, 512], f32, kind="ExternalOutput")
    if dump:
        hd_d = nc.dram_tensor("hdump", [CT, 128, 2, 512], bf16, kind="ExternalOutput")
        qkd_d = nc.dram_tensor("qkdump", [2, 128, 2, 512], bf16, kind="ExternalOutput")
        vtd_d = nc.dram_tensor("vtdump", [128, 8, 66], bf16, kind="ExternalOutput")
        pd_d = nc.dram_tensor("pdump", [128, 1024], bf16, kind="ExternalOutput")
        od_d = nc.dram_tensor("odump", [128, 512], f32, kind="ExternalOutput")
        rbd_d = nc.dram_tensor("rbdump", [64, 512], f32, kind="ExternalOutput")
        osd_d = nc.dram_tensor("osdump", [CT, 128, 2, 512], bf16, kind="ExternalOutput")

    with tile.TileContext(nc) as tc, ExitStack() as ctx:
        persist = ctx.enter_context(tc.tile_pool(name="persist", bufs=1))
        work = ctx.enter_context(tc.tile_pool(name="work", bufs=2))
        pwork = ctx.enter_context(tc.tile_pool(name="pwork", bufs=6))
        small = ctx.enter_context(tc.tile_pool(name="small", bufs=4))
        psp = ctx.enter_context(tc.tile_pool(name="psp", bufs=2, space="PSUM"))
        dscr = ctx.enter_context(tc.tile_pool(name="dscr", bufs=4, space="DRAM"))

        # ---- x + groupnorm constants first, weights after ----
        x_sb = [persist.tile([128, 2, 512], f32, name=f"x{t}", tag=f"x{t}") for t in range(CT)]
        for t in range(CT):
            (nc.sync if t % 2 == 0 else nc.gpsimd).dma_start(out=x_sb[t], in_=x_d.ap()[t])
        g_sb = persist.tile([128, 4, 8], f32, tag="gmat")
        nc.gpsimd.dma_start(out=g_sb, in_=g_d.ap())
        r_sb = persist.tile([8, 4, 128], f32, tag="rmat")
        nc.gpsimd.dma_start(out=r_sb, in_=r_d.ap())
        gnw_sb = persist.tile([128, 4], f32, tag="gnw")
        nc.gpsimd.dma_start(out=gnw_sb, in_=gnw_d.ap())
        gnb_sb = persist.tile([128, 4], f32, tag="gnb")
        nc.gpsimd.dma_start(out=gnb_sb, in_=gnb_d.ap())

        wqk_sb = [persist.tile([128, 1024], bf16, name=f"wqk{t}", tag=f"wqk{t}") for t in range(CT)]
        wv_sb = [persist.tile([128, 512], bf16, name=f"wv{t}", tag=f"wv{t}") for t in range(CT)]
        wp_sb = [persist.tile([128, 512], bf16, name=f"wp{t}", tag=f"wp{t}") for t in range(CT)]
        for t in range(CT):
            nc.sync.dma_start(out=wqk_sb[t], in_=wqk_d.ap()[t])
            nc.gpsimd.dma_start(out=wv_sb[t], in_=wv_d.ap()[t])
        qkb_sb = persist.tile([128, 8], f32, tag="qkb")
        nc.sync.dma_start(out=qkb_sb, in_=qkb_d.ap())
        for t in range(CT):
            nc.gpsimd.dma_start(out=wp_sb[t], in_=wp_d.ap()[t])
        pb_sb = persist.tile([128, 4], f32, tag="pb")
        nc.gpsimd.dma_start(out=pb_sb, in_=pb_d.ap())
        id_sb = persist.tile([128, 128], bf16, tag="ident")
        nc.gpsimd.dma_start(out=id_sb, in_=id_d.ap())
        # bf16 copy of x for the fused proj residual (gpsimd DMA casts)
        xb_sb = [persist.tile([128, 2, 512], bf16, name=f"xb{t}", tag=f"xb{t}") for t in range(CT)]
        for t in range(CT):
            nc.gpsimd.dma_start(out=xb_sb[t], in_=x_d.ap()[t])

        ones_f32 = persist.tile([1, 64], f32, tag="ones_f32")
        nc.vector.memset(ones_f32, 1.0)
        eps_sb = persist.tile([8, 1], f32, tag="eps")
        nc.vector.memset(eps_sb, EPS)
        # preload the ln/exp ACT table set while DMAs are in flight
        dummy = persist.tile([1, 1], f32, tag="dummy")
        nc.scalar.activation(out=dummy, in_=ones_f32[:, 0:1], func=AF.Ln)

        # ---- GroupNorm + h, fully per-c-tile (pipelines with x DMA) ----
        h_sb = [persist.tile([128, 2, 512], bf16, name=f"h{t}", tag=f"h{t}") for t in range(CT)]
        for t in range(CT):
            st = small.tile([128, 2, 6], f32, tag="bnst")
            for s in range(2):
                nc.vector.bn_stats(out=st[:, s, :], in_=x_sb[t][:, s, :])
            mv = small.tile([128, 2], f32, tag="mv")
            nc.vector.bn_aggr(out=mv, in_=st)
            # s2[:, 0] = mean_c ; s2[:, 1] = E[x^2]_c
            s2 = small.tile([128, 2], f32, tag="s2")
            nc.vector.tensor_copy(out=s2[:, 0:1], in_=mv[:, 0:1])
            sq = small.tile([128, 1], f32, tag="sq")
            nc.vector.tensor_mul(out=sq, in0=mv[:, 0:1], in1=mv[:, 0:1])
            nc.vector.tensor_add(out=s2[:, 1:2], in0=sq, in1=mv[:, 1:2])
            # this tile's 8 groups: gst8 = (1/16) * sum_{c in g} (mean, E2)
            g8_ps = psp.tile([8, 2], f32, tag="big")
            nc.tensor.matmul(g8_ps, lhsT=g_sb[:, t, :], rhs=s2, start=True, stop=True)
            gst = small.tile([8, 2], f32, tag="gst")
            nc.vector.tensor_copy(out=gst, in_=g8_ps)
            gm2 = small.tile([8, 1], f32, tag="gm2")
            nc.vector.tensor_mul(out=gm2, in0=gst[:, 0:1], in1=gst[:, 0:1])
            gvar = small.tile([8, 1], f32, tag="gvar")
            nc.vector.tensor_sub(out=gvar, in0=gst[:, 1:2], in1=gm2)
            glv = small.tile([8, 1], f32, tag="glv")
            nc.scalar.activation(out=glv, in_=gvar, func=AF.Ln, bias=eps_sb, scale=1.0)
            gb = small.tile([8, 2], f32, tag="gb")
            nc.vector.tensor_copy(out=gb[:, 0:1], in_=gst[:, 0:1])
            # rstd = exp(-0.5 * ln(var + eps))
            nc.scalar.activation(out=gb[:, 1:2], in_=glv, func=AF.Exp, scale=-0.5)
            # broadcast group (mean, rstd) to the tile's 128 channels
            cb_ps = psp.tile([128, 2], f32, tag="big")
            nc.tensor.matmul(cb_ps, lhsT=r_sb[:, t, :], rhs=gb, start=True, stop=True)
            a_sb = small.tile([128, 1], f32, tag="gnA")
            nc.vector.tensor_mul(out=a_sb, in0=cb_ps[:, 1:2], in1=gnw_sb[:, t : t + 1])
            tb = small.tile([128, 1], f32, tag="gnT")
            nc.vector.tensor_mul(out=tb, in0=cb_ps[:, 0:1], in1=a_sb)
            b_sb = small.tile([128, 1], f32, tag="gnB")
            nc.vector.tensor_sub(out=b_sb, in0=gnb_sb[:, t : t + 1], in1=tb)
            nc.vector.tensor_scalar(
                out=h_sb[t], in0=x_sb[t], scalar1=a_sb, scalar2=b_sb,
                op0=OP.mult, op1=OP.add,
            )

        # ---- q,k / v^T / attention, interleaved so ScalarE exps start early.
        # PE program order: qk(0), qk(4), [pair0: S + vT inline + o], qk(1),
        # qk(5), [pair1], ... so the PE never sits behind work whose inputs
        # aren't ready, and exp (the bottleneck) is fed from ~t=10us on.
        # All PSUM lives in one [128, 1024] x4 tag: per pair 2 slots hold the
        # o accumulators, 2 cycle S tiles; between pairs qk groups cycle. ----
        qk_sb = [persist.tile([128, 2, 512], bf16, name=f"qk{m}", tag=f"qk{m}") for m in range(8)]
        vt_sb = [persist.tile([128, 8, 66], bf16, name=f"vt{i}", tag=f"vt{i}") for i in range(8)]
        o_sb = [persist.tile([128, 2, 512], bf16, name=f"o{hp}", tag=f"o{hp}") for hp in range(4)]

        def emit_qk(m):
            for nh in range(2):
                ps = psp.tile([128, 1024], f32, tag="big", name=f"qkp{m}_{nh}")
                for t in range(CT):
                    nc.tensor.matmul(
                        ps[:, 0:512], lhsT=wqk_sb[t][:, m * 128 : (m + 1) * 128],
                        rhs=h_sb[t][:, nh, :], start=(t == 0), stop=(t == CT - 1),
                    )
                nc.vector.tensor_scalar(
                    out=qk_sb[m][:, nh, :], in0=ps[:, 0:512],
                    scalar1=qkb_sb[:, m : m + 1], scalar2=None, op0=OP.add,
                )

        def emit_vt(i):
            nc.vector.memset(vt_sb[i][:, :, 64:66], 1.0)
            ps = psp.tile([128, 1024], f32, tag="big", name=f"vtp{i}")
            for t in range(CT):
                nc.tensor.matmul(
                    ps[:, 0:512],
                    lhsT=h_sb[t][:, i // 4, (i % 4) * 128 : (i % 4 + 1) * 128],
                    rhs=wv_sb[t], start=(t == 0), stop=(t == CT - 1),
                )
            nc.vector.tensor_copy(
                out=vt_sb[i][:, :, 0:64],
                in_=ps[:, 0:512].rearrange("p (h d) -> p h d", h=8),
            )

        emit_qk(0)
        emit_qk(4)
        if dump:
            for t in range(CT):
                nc.sync.dma_start(out=hd_d.ap()[t], in_=h_sb[t])
            nc.sync.dma_start(out=qkd_d.ap()[0], in_=qk_sb[0])
            nc.sync.dma_start(out=qkd_d.ap()[1], in_=qk_sb[4])

        def emit_qk_group(m, nh):
            ps = psp.tile([128, 1024], f32, tag="big", name=f"qkp{m}_{nh}")
            for t in range(CT):
                nc.tensor.matmul(
                    ps[:, 0:512], lhsT=wqk_sb[t][:, m * 128 : (m + 1) * 128],
                    rhs=h_sb[t][:, nh, :], start=(t == 0), stop=(t == CT - 1),
                )
            nc.vector.tensor_scalar(
                out=qk_sb[m][:, nh, :], in0=ps[:, 0:512],
                scalar1=qkb_sb[:, m : m + 1], scalar2=None, op0=OP.add,
            )

        pending_tail = []
        s_t = {}

        def emit_s(hp, mi):
            for hh in range(2):
                sp = psp.tile([128, 1024], f32, tag="big", name=f"s{hp}_{mi}_{hh}")
                s_t[(hp, mi, hh)] = sp
                po = 64 * hh
                for nh in range(2):
                    nc.tensor.matmul(
                        sp[:, nh * 512 : (nh + 1) * 512],
                        lhsT=qk_sb[4 + hp][po : po + 64, mi // 4, (mi % 4) * 128 : (mi % 4 + 1) * 128],
                        rhs=qk_sb[hp][po : po + 64, nh, :],
                        start=True, stop=True,
                    )

        emit_s(0, 0)
        emit_vt(0)
        emit_vt(1)
        for hp in range(4):
            o_ps = {}
            for hh in range(2):
                for nh in range(2):
                    o_ps[(hh, nh)] = psp.tile(
                        [128, 512], f32, tag="obank", bufs=4, name=f"ops{hp}_{hh}{nh}"
                    )
            for mi in range(8):
                if mi < 7:
                    emit_s(hp, mi + 1)
                elif hp < 3:
                    emit_s(hp + 1, 0)
                if hp == 0 and mi < 6:
                    emit_vt(mi + 2)
                # drain the previous pair's normalization tail under this
                # pair's exp stream (PE slack), one (hh, nh) chunk per step
                if mi < 4 and pending_tail:
                    pending_tail.pop(0)()
                # prefetch the next pair's q/k under this pair's exp stream;
                # the current pair's k nh1-half (only needed from S(hp,4),
                # emitted at step 3) arrives at step 0 to even out PE load
                if hp > 0 and mi == 0:
                    emit_qk_group(4 + hp, 1)
                if hp < 3 and 4 <= mi <= 6:
                    m = (hp + 1) if mi < 6 else (hp + 5)
                    emit_qk_group(m, mi % 2)
                for hh in range(2):
                    pt = pwork.tile([128, 1024], bf16, tag="pt", bufs=12)
                    nc.scalar.activation(
                        out=pt, in_=s_t.pop((hp, mi, hh)), func=AF.Exp, scale=SCALE,
                    )
                    if dump and hp == 0 and mi == 0 and hh == 0:
                        nc.sync.dma_start(out=pd_d.ap(), in_=pt)
                    for nh in range(2):
                        nc.tensor.matmul(
                            o_ps[(hh, nh)][0:65, :],
                            lhsT=vt_sb[mi][:, 2 * hp + hh, 0:65],
                            rhs=pt[:, nh * 512 : (nh + 1) * 512],
                            start=(mi == 0), stop=(mi == 7),
                            skip_group_check=True,
                        )

            if dump and hp == 0:
                odt = work.tile([128, 512], f32, tag="odt")
                nc.vector.tensor_copy(out=odt, in_=o_ps[(0, 0)])
                nc.sync.dma_start(out=od_d.ap(), in_=odt)
                nc.sync.dma_start(out=vtd_d.ap(), in_=vt_sb[0])

            def make_tail(hp, hh, nh, op, last=(hp == 3)):
                def tail():
                    import concourse.bass as bass

                    po = 64 * hh
                    sm = small.tile([1, 512], f32, tag="sm", bufs=6, name=f"sm{hp}_{hh}{nh}")
                    if last:
                        # ScalarE is idle in the tail phase; keep DVE short
                        nc.scalar.activation(out=sm, in_=op[64:65, :], func=AF.Copy)
                    else:
                        nc.vector.tensor_copy(out=sm, in_=op[64:65, :])
                    rc = small.tile([1, 512], f32, tag="rc", bufs=6, name=f"rc{hp}_{hh}{nh}")
                    nc.vector.reciprocal_approx_fast(out=rc, in_=sm)
                    # broadcast 1/sums across 64 partitions: bounce through a
                    # DRAM scratch row, then stride-0 DMA back
                    dma_eng = (nc.sync if (hh + nh) % 2 == 0 else nc.gpsimd) if last else nc.gpsimd
                    scr = dscr.tile([1, 512], f32, tag="scr", name=f"scr{hp}_{hh}{nh}")
                    dma_eng.dma_start(out=scr, in_=rc)
                    rb = small.tile([64, 512], f32, tag="rb", bufs=6, name=f"rb{hp}_{hh}{nh}")
                    scr_bcast = bass.AP(
                        tensor=scr.tensor, offset=scr.offset,
                        ap=[[0, 64]] + list(scr.ap[1:]),
                    )
                    dma_eng.dma_start(out=rb, in_=scr_bcast)
                    nc.vector.tensor_mul(
                        out=o_sb[hp][po : po + 64, nh, :],
                        in0=op[0:64, :], in1=rb,
                    )
                    if dump and hp == 0 and hh == 0 and nh == 0:
                        nc.sync.dma_start(out=rbd_d.ap(), in_=rb)
                return tail

            for nh in range(2):
                for hh in range(2):
                    pending_tail.append(make_tail(hp, hh, nh, o_ps[(hh, nh)]))

        if dump:
            for t in range(CT):
                nc.sync.dma_start(out=osd_d.ap()[t], in_=o_sb[t])

        # ---- proj + residual (draining the last pair's tail first) ----
        for m in range(CT):
            ot = work.tile([128, 2, 512], f32, tag="ot")
            for nh in range(2):
                for _ in range(2):
                    if pending_tail:
                        pending_tail.pop(0)()
                ps = psp.tile([128, 1024], f32, tag="big", name=f"pj{m}_{nh}")
                for t in range(CT):
                    nc.tensor.matmul(
                        ps[:, 0:512], lhsT=wp_sb[t][:, m * 128 : (m + 1) * 128],
                        rhs=o_sb[t][:, nh, :], start=(t == 0), stop=False,
                    )
                nc.tensor.matmul(
                    ps[:, 0:512], lhsT=id_sb, rhs=xb_sb[m][:, nh, :],
                    start=False, stop=True,
                )
                nc.scalar.activation(
                    out=ot[:, nh, :], in_=ps[:, 0:512], func=AF.Identity,
                    bias=pb_sb[:, m : m + 1], scale=1.0,
                )
                nc.sync.dma_start(out=out_d.ap()[m, :, nh, :], in_=ot[:, nh, :])

    nc.compile()
    return nc


def _prep_inputs(inputs):
    x = np.ascontiguousarray(np.asarray(inputs["x"], dtype=np.float32))
    gn_w = np.asarray(inputs["gn_weight"], dtype=np.float32)
    gn_b = np.asarray(inputs["gn_bias"], dtype=np.float32)
    qkv_w = np.asarray(inputs["qkv_weight"], dtype=np.float32)
    qkv_b = np.asarray(inputs["qkv_bias"], dtype=np.float32)
    p_w = np.asarray(inputs["proj_weight"], dtype=np.float32)
    p_b = np.asarray(inputs["proj_bias"], dtype=np.float32)

    wqkT = np.ascontiguousarray(qkv_w[:1024].T).reshape(CT, 128, 1024).astype(BF16)
    wvT = np.ascontiguousarray(qkv_w[1024:].T).reshape(CT, 128, 512).astype(BF16)
    wpT = np.ascontiguousarray(p_w.T).reshape(CT, 128, 512).astype(BF16)
    qkb = np.ascontiguousarray(qkv_b[:1024].reshape(8, 128).T)  # [128, 8]
    # v-bias enters o additively (softmax rows sum to 1), so it folds through
    # the projection into an effective proj bias: pb' = pb + Wp @ vbias
    pb_eff = p_b + p_w.astype(np.float64) @ qkv_b[1024:].astype(np.float64)
    pb = np.ascontiguousarray(pb_eff.astype(np.float32).reshape(4, 128).T)  # [128, 4]
    gnw = np.ascontiguousarray(gn_w.reshape(4, 128).T)
    gnb = np.ascontiguousarray(gn_b.reshape(4, 128).T)

    # per-c-tile group-sum (G) and group-broadcast (R) selector matrices
    gmat = np.zeros((4, 128, 8), np.float32)
    rmat = np.zeros((8, 4, 128), np.float32)
    for t in range(4):
        for c in range(128):
            gmat[t, c, c // GS] = 1.0 / GS
            rmat[c // GS, t, c] = 1.0
    gmat = np.ascontiguousarray(gmat.transpose(1, 0, 2))        # [128, 4, 8]

    shared = dict(
        wqkT=wqkT, wvT=wvT, wpT=wpT, qkbias=qkb, pbias=pb,
        gnw=gnw, gnb=gnb, gmat=gmat, rmat=rmat,
        ident=np.eye(128, dtype=np.float32).astype(BF16),
    )
    xs = x.reshape(B, CT, 128, 2, 512)
    in_maps = [dict(shared, x=np.ascontiguousarray(xs[b])) for b in range(B)]
    return in_maps


def _get_nc(dump=False):
    key = ("ncd" if dump else "nc")
    if key not in _CACHE:
        _CACHE[key] = _build_nc(dump)
    return _CACHE[key]


def _run(inputs, trace=False):
    from concourse import bass_utils

    nc = _get_nc()
    in_maps = _prep_inputs(inputs)
    res = bass_utils.run_bass_kernel_spmd(
        nc, in_maps, core_ids=list(range(B)), trace=trace,
    )
    out = np.stack([r["out"].reshape(C, HH, WW) for r in res.results])
    return out.astype(np.float32), res


def kernel(**inputs) -> np.ndarray:
    out, _ = _run(inputs, trace=False)
    return out

